# revision 1
# baseline (speedup 1.0000x reference)
"""GCN 2-layer message passing on 8 Trainium2 NeuronCores.

Strategy (graph/data parallel, hardcoded for N=100000, E=1600000, 128->64->32):
  - Nodes sharded by contiguous range across 8 cores (12544 rows/core, padded).
  - Symmetric normalization folded into per-node row scales (dinv), applied
    pre/post aggregation -> per-edge work is pure gather + segment-sum.
  - Edges owned by dst core, grouped into 128-node dst groups; blocks of 6
    groups x 4 src chunks (int16 gather index limit) form one dma_gather call
    each, UNPADDED (per-call num_idxs = max real edge count across cores; the
    SWDGE descriptor generation on GpSimd is the bottleneck at ~8ns/descriptor,
    so padded descriptors are the thing to avoid).
  - Gathered 128-edge tiles may straddle dst groups: each (tile, group) pair
    present on any core gets its own selection column; P[edge, slot] =
    (dst_rel == iota) built on VectorE (non-members/-1 never match), and
    TensorE matmul-accumulates P.T @ msg into the group's PSUM tile.
    This replaces scatter-add entirely; duplicates merge in PSUM.
  - GCN's added self-loops never enter the edge lists; each group gets one
    identity matmul adding its own rows (kept resident in SBUF).
  - AllGather (collective) re-replicates the per-core node tables for the
    matmul input (u1) and at the layer boundary (u2).
"""
import sys

sys.path.insert(0, "/opt/trn_rl_repo")

import numpy as np
import ml_dtypes

from concourse import bass, mybir
import concourse.bacc as bacc
import concourse.tile as tile
from concourse import bass_utils

BF16 = ml_dtypes.bfloat16

NCORES = 8
N = 100000
IN_CH = 128
HID = 64
OUT_CH = 32
SLICE = 12544          # nodes per core (98 groups of 128)
NPAD = SLICE * NCORES  # 100352
G = SLICE // 128       # 98 groups per core
NCHUNK = 4
CHUNK = NPAD // NCHUNK  # 25088 (< 32768, int16-addressable)
BLOCK = 6              # dst groups per block (PSUM bank budget)
FEAT = 128             # padded bf16 row width of node tables (256B rows)
MSGBUFS = 3
PAD_NEG = False  # pad idx tails with -1 (SWDGE truncates, skipping emission)


def configure(n):
    """Set problem size (test hook). Recomputes sharding constants."""
    global N, SLICE, NPAD, G, CHUNK
    N = n
    SLICE = -(-N // (NCORES * 128)) * 128
    NPAD = SLICE * NCORES
    G = SLICE // 128
    CHUNK = NPAD // NCHUNK
    assert CHUNK % 16 == 0 and CHUNK < 32768


# ----------------------------------------------------------------------------
# host-side preprocessing: sharding, schedule, index arrays
# ----------------------------------------------------------------------------

def _host_prep(x, edge_index, W1, b1, W2, b2):
    src = edge_index[0].astype(np.int64)
    dst = edge_index[1].astype(np.int64)
    # degree includes the GCN-added self loop (handled on-device as identity)
    deg = (np.bincount(dst, minlength=N) + 1).astype(np.float32)
    dinv = (1.0 / np.sqrt(deg)).astype(np.float32)

    core = (dst // SLICE).astype(np.int64)          # dst owner
    g_loc = ((dst - core * SLICE) // 128).astype(np.int64)
    blk = g_loc // BLOCK
    # src chunk q holds local rows [q*SLICE/4,(q+1)*SLICE/4) of every core,
    # so each chunk is filled by its own (pipelined) AllGather
    qsz = SLICE // NCHUNK
    c_src = src // SLICE
    l_src = src - c_src * SLICE
    ch = l_src // qsz
    nblocks = -(-G // BLOCK)
    call_of = blk * NCHUNK + ch                     # call id within core
    ncalls = nblocks * NCHUNK
    dst_rel = (dst - core * SLICE - g_loc * 128).astype(np.int32)
    idx16 = (c_src * qsz + (l_src - ch * qsz)).astype(np.int16)

    # sort edges by (core, call, group) so group runs are contiguous per call
    key = (core * ncalls + call_of) * G + g_loc
    order = np.argsort(key, kind="stable")
    cc_s = (core * ncalls + call_of)[order]
    g_s = g_loc[order].astype(np.int32)
    idx16_s = idx16[order]
    dstrel_s = dst_rel[order]

    counts = np.bincount(cc_s, minlength=NCORES * ncalls).reshape(
        NCORES, ncalls)
    starts = np.zeros(NCORES * ncalls + 1, np.int64)
    np.cumsum(counts.reshape(-1), out=starts[1:])
    nidx_call = counts.max(axis=0)                  # [ncalls]
    ntile_call = -(-nidx_call // 128)

    # per-core per-call slot arrays (group id, dstrel, idx), padded to max
    # count with (g=-1, dstrel=-1, idx=0)
    mm_lists = []          # per call: ordered [(tile, group), ...]
    for ci in range(ncalls):
        nt = int(ntile_call[ci])
        pairs = set()
        for c in range(NCORES):
            lo, hi = starts[c * ncalls + ci], starts[c * ncalls + ci + 1]
            gs = g_s[lo:hi]
            for t in range(nt):
                for g in np.unique(gs[t * 128:(t + 1) * 128]):
                    pairs.add((t, int(g)))
        mm_lists.append(sorted(pairs))
    nmm = sum(len(m) for m in mm_lists)
    ntiles = int(ntile_call.sum())
    nidx_tot = int(nidx_call.sum())
    idx_cols = [-(-int(n) // 16) for n in nidx_call]
    nidx_coltot = sum(idx_cols)

    idx_w = np.zeros((NCORES, 128, nidx_coltot), np.int16)
    drel_w = np.full((NCORES, 128, nmm), -1.0, np.float32)
    for c in range(NCORES):
        mmoff = 0
        coloff = 0
        for ci in range(ncalls):
            nt = int(ntile_call[ci])
            ncap = nt * 128
            lo, hi = starts[c * ncalls + ci], starts[c * ncalls + ci + 1]
            n = hi - lo
            gs = np.full(ncap, -1, np.int32)
            drs = np.full(ncap, -1.0, np.float32)
            ids = np.full(ncap, -1 if PAD_NEG else 0, np.int16)
            gs[:n] = g_s[lo:hi]
            drs[:n] = dstrel_s[lo:hi]
            ids[:n] = idx16_s[lo:hi]
            # idx wrap for this call: i -> [i%16, i//16], replicated x8
            ni = int(nidx_call[ci])
            w16 = idx_cols[ci]
            blk16 = ids[:w16 * 16].reshape(w16, 16).T
            idx_w[c, :, coloff:coloff + w16] = np.tile(blk16, (8, 1))
            coloff += w16
            # selection columns per (tile, group)
            for j, (t, g) in enumerate(mm_lists[ci]):
                seg_g = gs[t * 128:(t + 1) * 128]
                seg_d = drs[t * 128:(t + 1) * 128]
                drel_w[c, :, mmoff + j] = np.where(seg_g == g, seg_d, -1.0)
            mmoff += len(mm_lists[ci])

    # per-core prescaled transposed features (bf16), zero padded
    xs = x * dinv[:, None]
    xT = np.zeros((NCORES, IN_CH, SLICE), BF16)
    dinv_w = np.zeros((NCORES, 128, G), np.float32)
    dinv2_w = np.zeros((NCORES, 128, G), np.float32)
    for c in range(NCORES):
        lo = c * SLICE
        hi = min(lo + SLICE, N)
        xT[c, :, :hi - lo] = xs[lo:hi].T.astype(BF16)
        dv = np.zeros(SLICE, np.float32)
        dv[:hi - lo] = dinv[lo:hi]
        dinv_w[c] = dv.reshape(G, 128).T
        dinv2_w[c] = (dv * dv).reshape(G, 128).T

    iota = np.tile(np.arange(128, dtype=np.float32), (128, 1)).astype(BF16)
    consts = {
        "w1_in": W1.astype(BF16),                            # [128, 64]
        "w2_in": W2.astype(BF16),                            # [64, 32]
        "b1_in": np.tile(b1.astype(np.float32), (128, 1)),   # [128, 64]
        "b2_in": np.tile(b2.astype(np.float32), (128, 1)),   # [128, 32]
        "iota_in": iota,
        "ident_in": np.eye(128, dtype=np.float32).astype(BF16),
    }
    in_maps = []
    for c in range(NCORES):
        m = dict(consts)
        m["xt_in"] = xT[c]
        m["idx_in"] = idx_w[c]
        m["drel_in"] = drel_w[c].astype(BF16)
        m["dinv_in"] = dinv_w[c]
        m["dinv2_in"] = dinv2_w[c]
        in_maps.append(m)

    sched = {
        "zero_bias": bool(np.all(b1 == 0) and np.all(b2 == 0)),
        "ncalls": ncalls,
        "nidx_call": [int(v) for v in nidx_call],
        "ntile_call": [int(v) for v in ntile_call],
        "idx_cols": idx_cols,
        "mm_lists": mm_lists,
        "nmm": nmm,
        "ntiles": ntiles,
        "nidx_coltot": nidx_coltot,
        "nblocks": nblocks,
    }
    return sched, in_maps


# ----------------------------------------------------------------------------
# device program
# ----------------------------------------------------------------------------

def _build_program(sched):
    f32 = mybir.dt.float32
    bf16 = mybir.dt.bfloat16
    ncalls = sched["ncalls"]
    mm_lists = sched["mm_lists"]
    nmm = sched["nmm"]
    nc = bacc.Bacc("TRN2", target_bir_lowering=False, debug=False,
                   num_devices=NCORES)

    xt = nc.dram_tensor("xt_in", [IN_CH, SLICE], bf16, kind="ExternalInput").ap()
    idx = nc.dram_tensor("idx_in", [128, sched["nidx_coltot"]], mybir.dt.int16,
                         kind="ExternalInput").ap()
    drel = nc.dram_tensor("drel_in", [128, nmm], bf16,
                          kind="ExternalInput").ap()
    dinv = nc.dram_tensor("dinv_in", [128, G], f32, kind="ExternalInput").ap()
    dinv2 = nc.dram_tensor("dinv2_in", [128, G], f32,
                           kind="ExternalInput").ap()
    w1 = nc.dram_tensor("w1_in", [IN_CH, HID], bf16, kind="ExternalInput").ap()
    w2 = nc.dram_tensor("w2_in", [HID, OUT_CH], bf16, kind="ExternalInput").ap()
    b1 = nc.dram_tensor("b1_in", [128, HID], f32, kind="ExternalInput").ap()
    b2 = nc.dram_tensor("b2_in", [128, OUT_CH], f32, kind="ExternalInput").ap()
    iota_t = nc.dram_tensor("iota_in", [128, 128], bf16,
                            kind="ExternalInput").ap()
    ident = nc.dram_tensor("ident_in", [128, 128], bf16,
                           kind="ExternalInput").ap()
    out = nc.dram_tensor("out", [SLICE, OUT_CH], f32, kind="ExternalOutput").ap()

    # first mm (global index) per group, and flush call per group
    first = {}
    gmm = 0
    for ci in range(ncalls):
        for (t, g) in mm_lists[ci]:
            if g not in first:
                first[g] = gmm
            gmm += 1
    flush_ci = {}
    for g in range(G):
        bi = g // BLOCK
        flush_ci[g] = min((bi + 1) * NCHUNK, ncalls) - 1

    wmax = max(sched["ntile_call"]) if ncalls else 1

    with tile.TileContext(nc) as tc:
        with tc.tile_pool(name="dram", bufs=1, space="DRAM") as dram, \
             tc.tile_pool(name="const", bufs=1) as cst, \
             tc.tile_pool(name="pmat", bufs=3) as pp, \
             tc.tile_pool(name="flush", bufs=3) as fl, \
             tc.tile_pool(name="gpsum", bufs=BLOCK, space="PSUM") as gps, \
             tc.tile_pool(name="mpsum", bufs=2, space="PSUM") as mps:

            # ---- constants / persistent SBUF ----
            idx_sb = cst.tile([128, sched["nidx_coltot"]], mybir.dt.int16)
            nc.sync.dma_start(out=idx_sb[:], in_=idx[:])
            drel_sb = cst.tile([128, nmm], bf16)
            nc.sync.dma_start(out=drel_sb[:], in_=drel[:])
            dinv_sb = cst.tile([128, G], f32)
            nc.sync.dma_start(out=dinv_sb[:], in_=dinv[:])
            dinv2_sb = cst.tile([128, G], f32)
            nc.sync.dma_start(out=dinv2_sb[:], in_=dinv2[:])
            w1_sb = cst.tile([IN_CH, HID], bf16)
            nc.sync.dma_start(out=w1_sb[:], in_=w1[:])
            w2_sb = cst.tile([HID, OUT_CH], bf16)
            nc.sync.dma_start(out=w2_sb[:], in_=w2[:])
            b1_sb = cst.tile([128, HID], f32)
            nc.sync.dma_start(out=b1_sb[:], in_=b1[:])
            b2_sb = cst.tile([128, OUT_CH], f32)
            nc.sync.dma_start(out=b2_sb[:], in_=b2[:])
            iota_sb = cst.tile([128, 128], bf16)
            nc.sync.dma_start(out=iota_sb[:], in_=iota_t[:])
            ident_sb = cst.tile([128, 128], bf16)
            nc.sync.dma_start(out=ident_sb[:], in_=ident[:])
            u_own = cst.tile([128, G, HID], bf16)   # this core's table rows

            # persistent msg buffers (zeroed once: stale tail slots must not
            # hold NaN bit patterns; 0 * garbage-NaN would poison PSUM)
            msgs = []
            for i in range(MSGBUFS):
                mt = cst.tile([128, wmax, FEAT], bf16, name=f"msgbuf{i}")
                nc.vector.memset(mt[:], 0.0)
                msgs.append(mt)

            # DRAM node tables, split into row quarters so each quarter's
            # AllGather starts as soon as its rows are written
            qsz = SLICE // NCHUNK
            u_loc = [dram.tile([qsz, FEAT], bf16, name=f"u_loc{q}")
                     for q in range(NCHUNK)]
            u_fullA = [dram.tile([CHUNK, FEAT], bf16, name=f"u_fullA{q}")
                       for q in range(NCHUNK)]
            u_fullB = [dram.tile([CHUNK, FEAT], bf16, name=f"u_fullB{q}")
                       for q in range(NCHUNK)]

            def write_rows(src_ap, g):
                # DMA u_own[:, g, :]-style tile rows [g*128,(g+1)*128) into
                # the quarter tiles (a group can span two quarters)
                r0 = g * 128
                p = 0
                while p < 128:
                    q = (r0 + p) // qsz
                    take = min(128 - p, (q + 1) * qsz - (r0 + p))
                    nc.sync.dma_start(
                        out=u_loc[q][r0 + p - q * qsz:
                                     r0 + p - q * qsz + take, 0:HID],
                        in_=src_ap[p:p + take])
                    p += take

            # ---- phase A: u1 = (dinv*x) @ W1, local rows ----
            with tc.tile_pool(name="xt", bufs=1) as xtp:
                xt_sb = xtp.tile([IN_CH, SLICE], bf16)
                nc.sync.dma_start(out=xt_sb[:], in_=xt[:])
                for g in range(G):
                    u1_ps = mps.tile([128, HID], f32, space="PSUM",
                                     tag="mps", name=f"u1ps_{g}")
                    nc.tensor.matmul(out=u1_ps[:],
                                     lhsT=xt_sb[:, g * 128:(g + 1) * 128],
                                     rhs=w1_sb[:], start=True, stop=True)
                    nc.scalar.activation(
                        out=u_own[:, g, :], in_=u1_ps[:],
                        func=mybir.ActivationFunctionType.Copy)
                    write_rows(u_own[:, g, :], g)

            def allgather(dst):
                for q in range(NCHUNK):
                    nc.gpsimd.collective_compute(
                        "AllGather", mybir.AluOpType.bypass,
                        replica_groups=[list(range(NCORES))],
                        ins=[u_loc[q][:].opt()], outs=[dst[q][:].opt()],
                    )

            zero_bias = sched["zero_bias"]

            def _flush(lname, g, ps, final):
                if not final:
                    # self loop: psum += I.T @ u_own[g]
                    nc.tensor.matmul(out=ps[:], lhsT=ident_sb[:],
                                     rhs=u_own[:, g, :],
                                     start=(g not in first), stop=True)
                    dv = dinv_sb[:, g:g + 1]
                    if zero_bias:
                        # dinv>0: dinv*relu(dinv*psum) == relu(dinv^2*psum).
                        # One ScalarE op; keeps VectorE free (it stalls badly
                        # against concurrent SWDGE descriptor generation).
                        nc.scalar.activation(
                            out=u_own[:, g, :], in_=ps[:],
                            func=mybir.ActivationFunctionType.Relu,
                            scale=dinv2_sb[:, g:g + 1])
                    else:
                        t1 = fl.tile([128, HID], f32, tag="f1",
                                     name=f"{lname}t1_{g}")
                        nc.vector.tensor_scalar(
                            out=t1[:], in0=ps[:], scalar1=dv, scalar2=None,
                            op0=mybir.AluOpType.mult)
                        nc.vector.tensor_tensor(
                            out=t1[:], in0=t1[:], in1=b1_sb[:],
                            op=mybir.AluOpType.add)
                        t2 = fl.tile([128, HID], f32, tag="f2",
                                     name=f"{lname}t2_{g}")
                        nc.scalar.activation(
                            out=t2[:], in_=t1[:],
                            func=mybir.ActivationFunctionType.Relu)
                        nc.vector.tensor_scalar(
                            out=u_own[:, g, :], in0=t2[:], scalar1=dv,
                            scalar2=None, op0=mybir.AluOpType.mult)
                    write_rows(u_own[:, g, :], g)
                else:
                    # self loop (transposed): psumT += u_own[g].T
                    nc.tensor.matmul(out=ps[:], lhsT=u_own[:, g, :],
                                     rhs=ident_sb[:],
                                     start=(g not in first), stop=True)
                    # aggT @ W2, then row-scale by dinv (diagonal commutes)
                    aggT = fl.tile([HID, 128], bf16, tag="f1",
                                   name=f"{lname}aggT_{g}")
                    nc.scalar.activation(
                        out=aggT[:], in_=ps[:],
                        func=mybir.ActivationFunctionType.Copy)
                    o_ps = mps.tile([128, OUT_CH], f32, space="PSUM",
                                    tag="mps", name=f"{lname}ops_{g}")
                    nc.tensor.matmul(out=o_ps[:], lhsT=aggT[:], rhs=w2_sb[:],
                                     start=True, stop=True)
                    o_sb = fl.tile([128, OUT_CH], f32, tag="f3",
                                   name=f"{lname}osb_{g}")
                    if zero_bias:
                        nc.scalar.activation(
                            out=o_sb[:], in_=o_ps[:],
                            func=mybir.ActivationFunctionType.Copy,
                            scale=dinv_sb[:, g:g + 1])
                    else:
                        nc.vector.tensor_scalar(
                            out=o_sb[:], in0=o_ps[:],
                            scalar1=dinv_sb[:, g:g + 1],
                            scalar2=None, op0=mybir.AluOpType.mult)
                        nc.vector.tensor_tensor(
                            out=o_sb[:], in0=o_sb[:], in1=b2_sb[:],
                            op=mybir.AluOpType.add)
                    nc.sync.dma_start(
                        out=out[g * 128:(g + 1) * 128, :], in_=o_sb[:])

            def layer(lname, final, ufull):
                psum = {}
                coloff = 0
                mmoff = 0
                for ci in range(ncalls):
                    ch = ci % NCHUNK
                    ni = sched["nidx_call"][ci]
                    nt = sched["ntile_call"][ci]
                    w16 = sched["idx_cols"][ci]
                    mml = mm_lists[ci]
                    if ni == 0:
                        coloff += w16
                        mmoff += len(mml)
                        continue
                    msg = msgs[ci % MSGBUFS]
                    nc.gpsimd.dma_gather(
                        out_ap=msg[:, 0:nt, :],
                        in_ap=ufull[ch][:],
                        idxs_ap=idx_sb[:, coloff:coloff + w16],
                        num_idxs=ni, num_idxs_reg=ni,
                        elem_size=FEAT, single_packet=False,
                    )
                    nmm_c = len(mml)
                    pm = pp.tile([128, nmm_c, 128], bf16, tag="pmat",
                                 name=f"{lname}pm_{ci}")
                    nc.vector.tensor_tensor(
                        out=pm[:],
                        in0=drel_sb[:, mmoff:mmoff + nmm_c]
                            .to_broadcast([128, nmm_c, 128]),
                        in1=iota_sb[:].unsqueeze(1)
                            .to_broadcast([128, nmm_c, 128]),
                        op=mybir.AluOpType.is_equal,
                    )
                    for j, (t, g) in enumerate(mml):
                        if g not in psum:
                            shape = [HID, 128] if final else [128, HID]
                            psum[g] = gps.tile(shape, f32, space="PSUM",
                                               tag="gacc",
                                               name=f"{lname}acc_{g}")
                        gm = mmoff + j
                        if final:
                            nc.tensor.matmul(
                                out=psum[g][:],
                                lhsT=msg[:, t, 0:HID],
                                rhs=pm[:, j, :],
                                start=(gm == first[g]), stop=False)
                        else:
                            nc.tensor.matmul(
                                out=psum[g][:],
                                lhsT=pm[:, j, :],
                                rhs=msg[:, t, 0:HID],
                                start=(gm == first[g]), stop=False)
                    coloff += w16
                    mmoff += len(mml)
                    # flush groups whose block ends at this call
                    for g in sorted(k for k, v in flush_ci.items() if v == ci):
                        if g not in psum:
                            shape = [HID, 128] if final else [128, HID]
                            psum[g] = gps.tile(shape, f32, space="PSUM",
                                               tag="gacc",
                                               name=f"{lname}acc_{g}")
                        _flush(lname, g, psum.pop(g), final)

            allgather(u_fullA)          # u1
            layer("L1", final=False, ufull=u_fullA)
            allgather(u_fullB)          # u2 (overlaps L1 tail: no WAR on A)
            layer("L2", final=True, ufull=u_fullB)

    nc.compile()
    return nc


_CACHE = {}


def kernel(x, edge_index, W1, b1, W2, b2):
    x = np.asarray(x, np.float32)
    edge_index = np.asarray(edge_index, np.int64)
    sched, in_maps = _host_prep(
        x, edge_index, np.asarray(W1, np.float32), np.asarray(b1, np.float32),
        np.asarray(W2, np.float32), np.asarray(b2, np.float32))
    key = (sched["nmm"], sched["ntiles"], sched["nidx_coltot"],
           sched["zero_bias"])
    if key not in _CACHE:
        _CACHE[key] = _build_program(sched)
    nc = _CACHE[key]
    res = bass_utils.run_bass_kernel_spmd(nc, in_maps,
                                          core_ids=list(range(NCORES)))
    outs = []
    for c in range(NCORES):
        lo = c * SLICE
        hi = min(lo + SLICE, N)
        outs.append(res.results[c]["out"][:hi - lo])
    return np.concatenate(outs, 0).astype(np.float32)



# revision 6
# speedup vs baseline: 2.5055x; 2.5055x over previous
"""GCN 2-layer message passing on 8 Trainium2 NeuronCores.

Strategy (graph/data parallel, hardcoded for N=100000, E=1600000, 128->64->32):
  - Nodes sharded by contiguous range across 8 cores (12544 rows/core, padded).
  - Symmetric normalization folded into per-node row scales (dinv), applied
    pre/post aggregation -> per-edge work is pure gather + segment-sum.
  - Edges owned by dst core, grouped into 128-node dst groups; blocks of 6
    groups x 4 src chunks (int16 gather index limit) form one dma_gather call
    each, UNPADDED (per-call num_idxs = max real edge count across cores; the
    SWDGE descriptor generation on GpSimd is the bottleneck at ~8ns/descriptor,
    so padded descriptors are the thing to avoid).
  - Gathered 128-edge tiles may straddle dst groups: each (tile, group) pair
    present on any core gets its own selection column; P[edge, slot] =
    (dst_rel == iota) built on VectorE (non-members/-1 never match), and
    TensorE matmul-accumulates P.T @ msg into the group's PSUM tile.
    This replaces scatter-add entirely; duplicates merge in PSUM.
  - GCN's added self-loops never enter the edge lists; each group gets one
    identity matmul adding its own rows (kept resident in SBUF).
  - AllGather (collective) re-replicates the per-core node tables for the
    matmul input (u1) and at the layer boundary (u2).
"""
import sys

sys.path.insert(0, "/opt/trn_rl_repo")

import numpy as np
import ml_dtypes

from concourse import bass, mybir
import concourse.bacc as bacc
import concourse.tile as tile
from concourse import bass_utils

BF16 = ml_dtypes.bfloat16

NCORES = 8
N = 100000
IN_CH = 128
HID = 64
OUT_CH = 32
SLICE = 12544          # nodes per core (98 groups of 128)
NPAD = SLICE * NCORES  # 100352
G = SLICE // 128       # 98 groups per core
NCHUNK = 4
CHUNK = NPAD // NCHUNK  # 25088 (< 32768, int16-addressable)
BLOCK = 6              # dst groups per block (PSUM bank budget)
FEAT = 128             # padded bf16 row width of node tables (256B rows)
MSGBUFS = 6
NQUEUES = 4            # SWDGE queues; queue q's desc-gen runs on DSP pair q
PAD_NEG = False  # pad idx tails with -1 (breaks static ring accounting!)


def configure(n):
    """Set problem size (test hook). Recomputes sharding constants."""
    global N, SLICE, NPAD, G, CHUNK
    N = n
    SLICE = -(-N // (NCORES * 128)) * 128
    NPAD = SLICE * NCORES
    G = SLICE // 128
    CHUNK = NPAD // NCHUNK
    assert CHUNK % 16 == 0 and CHUNK < 32768


# ----------------------------------------------------------------------------
# host-side preprocessing: sharding, schedule, index arrays
# ----------------------------------------------------------------------------

def _host_prep(x, edge_index, W1, b1, W2, b2):
    src = edge_index[0].astype(np.int64)
    dst = edge_index[1].astype(np.int64)
    # degree includes the GCN-added self loop (handled on-device as identity)
    deg = (np.bincount(dst, minlength=N) + 1).astype(np.float32)
    dinv = (1.0 / np.sqrt(deg)).astype(np.float32)

    core = (dst // SLICE).astype(np.int64)          # dst owner
    g_loc = ((dst - core * SLICE) // 128).astype(np.int64)
    blk = g_loc // BLOCK
    # src chunk q holds local rows [q*SLICE/4,(q+1)*SLICE/4) of every core,
    # so each chunk is filled by its own (pipelined) AllGather
    qsz = SLICE // NCHUNK
    c_src = src // SLICE
    l_src = src - c_src * SLICE
    ch = l_src // qsz
    nblocks = -(-G // BLOCK)
    call_of = blk * NCHUNK + ch                     # call id within core
    ncalls = nblocks * NCHUNK
    dst_rel = (dst - core * SLICE - g_loc * 128).astype(np.int32)
    idx16 = (c_src * qsz + (l_src - ch * qsz)).astype(np.int16)

    # sort edges by (core, call, group) so group runs are contiguous per call
    key = (core * ncalls + call_of) * G + g_loc
    order = np.argsort(key, kind="stable")
    cc_s = (core * ncalls + call_of)[order]
    g_s = g_loc[order].astype(np.int32)
    idx16_s = idx16[order]
    dstrel_s = dst_rel[order]

    counts = np.bincount(cc_s, minlength=NCORES * ncalls).reshape(
        NCORES, ncalls)
    starts = np.zeros(NCORES * ncalls + 1, np.int64)
    np.cumsum(counts.reshape(-1), out=starts[1:])
    nidx_call = counts.max(axis=0)                  # [ncalls]
    ntile_call = -(-nidx_call // 128)

    # per-core per-call slot arrays (group id, dstrel, idx), padded to max
    # count with (g=-1, dstrel=-1, idx=0)
    mm_lists = []          # per call: ordered [(tile, group), ...]
    for ci in range(ncalls):
        nt = int(ntile_call[ci])
        pairs = set()
        for c in range(NCORES):
            lo, hi = starts[c * ncalls + ci], starts[c * ncalls + ci + 1]
            gs = g_s[lo:hi]
            for t in range(nt):
                for g in np.unique(gs[t * 128:(t + 1) * 128]):
                    pairs.add((t, int(g)))
        mm_lists.append(sorted(pairs))
    nmm = sum(len(m) for m in mm_lists)
    ntiles = int(ntile_call.sum())
    nidx_tot = int(nidx_call.sum())
    idx_cols = [-(-int(n) // 16) for n in nidx_call]
    nidx_coltot = sum(idx_cols)

    idx_w = np.zeros((NCORES, 128, nidx_coltot), np.int16)
    drel_w = np.full((NCORES, 128, nmm), -1.0, np.float32)
    for c in range(NCORES):
        mmoff = 0
        coloff = 0
        for ci in range(ncalls):
            nt = int(ntile_call[ci])
            ncap = nt * 128
            lo, hi = starts[c * ncalls + ci], starts[c * ncalls + ci + 1]
            n = hi - lo
            gs = np.full(ncap, -1, np.int32)
            drs = np.full(ncap, -1.0, np.float32)
            ids = np.full(ncap, -1 if PAD_NEG else 0, np.int16)
            gs[:n] = g_s[lo:hi]
            drs[:n] = dstrel_s[lo:hi]
            ids[:n] = idx16_s[lo:hi]
            # idx wrap for this call: i -> [i%16, i//16], replicated x8
            ni = int(nidx_call[ci])
            w16 = idx_cols[ci]
            blk16 = ids[:w16 * 16].reshape(w16, 16).T
            idx_w[c, :, coloff:coloff + w16] = np.tile(blk16, (8, 1))
            coloff += w16
            # selection columns per (tile, group)
            for j, (t, g) in enumerate(mm_lists[ci]):
                seg_g = gs[t * 128:(t + 1) * 128]
                seg_d = drs[t * 128:(t + 1) * 128]
                drel_w[c, :, mmoff + j] = np.where(seg_g == g, seg_d, -1.0)
            mmoff += len(mm_lists[ci])

    # per-core prescaled transposed features (bf16), zero padded
    xs = x * dinv[:, None]
    xT = np.zeros((NCORES, IN_CH, SLICE), BF16)
    dinv_w = np.zeros((NCORES, 128, G), np.float32)
    dinv2_w = np.zeros((NCORES, 128, G), np.float32)
    for c in range(NCORES):
        lo = c * SLICE
        hi = min(lo + SLICE, N)
        xT[c, :, :hi - lo] = xs[lo:hi].T.astype(BF16)
        dv = np.zeros(SLICE, np.float32)
        dv[:hi - lo] = dinv[lo:hi]
        dinv_w[c] = dv.reshape(G, 128).T
        dinv2_w[c] = (dv * dv).reshape(G, 128).T

    iota = np.tile(np.arange(128, dtype=np.float32), (128, 1)).astype(BF16)
    consts = {
        "w1_in": W1.astype(BF16),                            # [128, 64]
        "w2_in": W2.astype(BF16),                            # [64, 32]
        "b1_in": np.tile(b1.astype(np.float32), (128, 1)),   # [128, 64]
        "b2_in": np.tile(b2.astype(np.float32), (128, 1)),   # [128, 32]
        "iota_in": iota,
        "ident_in": np.eye(128, dtype=np.float32).astype(BF16),
    }
    in_maps = []
    for c in range(NCORES):
        m = dict(consts)
        m["xt_in"] = xT[c]
        m["idx_in"] = idx_w[c]
        m["drel_in"] = drel_w[c].astype(BF16)
        m["dinv_in"] = dinv_w[c]
        m["dinv2_in"] = dinv2_w[c]
        in_maps.append(m)

    sched = {
        "zero_bias": bool(np.all(b1 == 0) and np.all(b2 == 0)),
        "ncalls": ncalls,
        "nidx_call": [int(v) for v in nidx_call],
        "ntile_call": [int(v) for v in ntile_call],
        "idx_cols": idx_cols,
        "mm_lists": mm_lists,
        "nmm": nmm,
        "ntiles": ntiles,
        "nidx_coltot": nidx_coltot,
        "nblocks": nblocks,
    }
    return sched, in_maps


# ----------------------------------------------------------------------------
# device program
# ----------------------------------------------------------------------------

def _build_program(sched):
    f32 = mybir.dt.float32
    bf16 = mybir.dt.bfloat16
    ncalls = sched["ncalls"]
    mm_lists = sched["mm_lists"]
    nmm = sched["nmm"]
    nc = bacc.Bacc("TRN2", target_bir_lowering=False, debug=False,
                   num_devices=NCORES, num_swdge_queues=NQUEUES)

    xt = nc.dram_tensor("xt_in", [IN_CH, SLICE], bf16, kind="ExternalInput").ap()
    idx = nc.dram_tensor("idx_in", [128, sched["nidx_coltot"]], mybir.dt.int16,
                         kind="ExternalInput").ap()
    drel = nc.dram_tensor("drel_in", [128, nmm], bf16,
                          kind="ExternalInput").ap()
    dinv = nc.dram_tensor("dinv_in", [128, G], f32, kind="ExternalInput").ap()
    dinv2 = nc.dram_tensor("dinv2_in", [128, G], f32,
                           kind="ExternalInput").ap()
    w1 = nc.dram_tensor("w1_in", [IN_CH, HID], bf16, kind="ExternalInput").ap()
    w2 = nc.dram_tensor("w2_in", [HID, OUT_CH], bf16, kind="ExternalInput").ap()
    b1 = nc.dram_tensor("b1_in", [128, HID], f32, kind="ExternalInput").ap()
    b2 = nc.dram_tensor("b2_in", [128, OUT_CH], f32, kind="ExternalInput").ap()
    iota_t = nc.dram_tensor("iota_in", [128, 128], bf16,
                            kind="ExternalInput").ap()
    ident = nc.dram_tensor("ident_in", [128, 128], bf16,
                           kind="ExternalInput").ap()
    out = nc.dram_tensor("out", [SLICE, OUT_CH], f32, kind="ExternalOutput").ap()

    # first mm (global index) per group, and flush call per group
    first = {}
    gmm = 0
    for ci in range(ncalls):
        for (t, g) in mm_lists[ci]:
            if g not in first:
                first[g] = gmm
            gmm += 1
    flush_ci = {}
    for g in range(G):
        bi = g // BLOCK
        flush_ci[g] = min((bi + 1) * NCHUNK, ncalls) - 1

    wmax = max(sched["ntile_call"]) if ncalls else 1

    with tile.TileContext(nc) as tc:
        with tc.tile_pool(name="dram", bufs=1, space="DRAM") as dram, \
             tc.tile_pool(name="const", bufs=1) as cst, \
             tc.tile_pool(name="pmat", bufs=3) as pp, \
             tc.tile_pool(name="flush", bufs=3) as fl, \
             tc.tile_pool(name="gpsum", bufs=BLOCK, space="PSUM") as gps, \
             tc.tile_pool(name="mpsum", bufs=2, space="PSUM") as mps:

            # ---- constants / persistent SBUF ----
            idx_sb = cst.tile([128, sched["nidx_coltot"]], mybir.dt.int16)
            nc.sync.dma_start(out=idx_sb[:], in_=idx[:])
            drel_sb = cst.tile([128, nmm], bf16)
            nc.sync.dma_start(out=drel_sb[:], in_=drel[:])
            dinv_sb = cst.tile([128, G], f32)
            nc.sync.dma_start(out=dinv_sb[:], in_=dinv[:])
            dinv2_sb = cst.tile([128, G], f32)
            nc.sync.dma_start(out=dinv2_sb[:], in_=dinv2[:])
            w1_sb = cst.tile([IN_CH, HID], bf16)
            nc.sync.dma_start(out=w1_sb[:], in_=w1[:])
            w2_sb = cst.tile([HID, OUT_CH], bf16)
            nc.sync.dma_start(out=w2_sb[:], in_=w2[:])
            b1_sb = cst.tile([128, HID], f32)
            nc.sync.dma_start(out=b1_sb[:], in_=b1[:])
            b2_sb = cst.tile([128, OUT_CH], f32)
            nc.sync.dma_start(out=b2_sb[:], in_=b2[:])
            iota_sb = cst.tile([128, 128], bf16)
            nc.sync.dma_start(out=iota_sb[:], in_=iota_t[:])
            ident_sb = cst.tile([128, 128], bf16)
            nc.sync.dma_start(out=ident_sb[:], in_=ident[:])
            u_own = cst.tile([128, G, HID], bf16)   # this core's table rows

            # persistent msg buffers (zeroed once: stale tail slots must not
            # hold NaN bit patterns; 0 * garbage-NaN would poison PSUM)
            msgs = []
            for i in range(MSGBUFS):
                mt = cst.tile([128, wmax, FEAT], bf16, name=f"msgbuf{i}")
                nc.vector.memset(mt[:], 0.0)
                msgs.append(mt)

            # DRAM node tables, split into row quarters so each quarter's
            # AllGather starts as soon as its rows are written
            qsz = SLICE // NCHUNK
            u_loc = [dram.tile([qsz, FEAT], bf16, name=f"u_loc{q}")
                     for q in range(NCHUNK)]
            u_fullA = [dram.tile([CHUNK, FEAT], bf16, name=f"u_fullA{q}")
                       for q in range(NCHUNK)]
            u_fullB = [dram.tile([CHUNK, FEAT], bf16, name=f"u_fullB{q}")
                       for q in range(NCHUNK)]

            def write_rows(src_ap, g):
                # DMA u_own[:, g, :]-style tile rows [g*128,(g+1)*128) into
                # the quarter tiles (a group can span two quarters)
                r0 = g * 128
                p = 0
                while p < 128:
                    q = (r0 + p) // qsz
                    take = min(128 - p, (q + 1) * qsz - (r0 + p))
                    nc.sync.dma_start(
                        out=u_loc[q][r0 + p - q * qsz:
                                     r0 + p - q * qsz + take, 0:HID],
                        in_=src_ap[p:p + take])
                    p += take

            # ---- phase A: u1 = (dinv*x) @ W1, local rows ----
            with tc.tile_pool(name="xt", bufs=1) as xtp:
                xt_sb = xtp.tile([IN_CH, SLICE], bf16)
                nc.sync.dma_start(out=xt_sb[:], in_=xt[:])
                for g in range(G):
                    u1_ps = mps.tile([128, HID], f32, space="PSUM",
                                     tag="mps", name=f"u1ps_{g}")
                    nc.tensor.matmul(out=u1_ps[:],
                                     lhsT=xt_sb[:, g * 128:(g + 1) * 128],
                                     rhs=w1_sb[:], start=True, stop=True)
                    nc.scalar.activation(
                        out=u_own[:, g, :], in_=u1_ps[:],
                        func=mybir.ActivationFunctionType.Copy)
                    write_rows(u_own[:, g, :], g)

            def allgather(dst):
                for q in range(NCHUNK):
                    nc.gpsimd.collective_compute(
                        "AllGather", mybir.AluOpType.bypass,
                        replica_groups=[list(range(NCORES))],
                        ins=[u_loc[q][:].opt()], outs=[dst[q][:].opt()],
                    )

            zero_bias = sched["zero_bias"]

            def _flush(lname, g, ps, final):
                if not final:
                    # self loop: psum += I.T @ u_own[g]
                    nc.tensor.matmul(out=ps[:], lhsT=ident_sb[:],
                                     rhs=u_own[:, g, :],
                                     start=(g not in first), stop=True)
                    dv = dinv_sb[:, g:g + 1]
                    if zero_bias:
                        # dinv>0: dinv*relu(dinv*psum) == relu(dinv^2*psum).
                        # One ScalarE op; keeps VectorE free (it stalls badly
                        # against concurrent SWDGE descriptor generation).
                        nc.scalar.activation(
                            out=u_own[:, g, :], in_=ps[:],
                            func=mybir.ActivationFunctionType.Relu,
                            scale=dinv2_sb[:, g:g + 1])
                    else:
                        t1 = fl.tile([128, HID], f32, tag="f1",
                                     name=f"{lname}t1_{g}")
                        nc.vector.tensor_scalar(
                            out=t1[:], in0=ps[:], scalar1=dv, scalar2=None,
                            op0=mybir.AluOpType.mult)
                        nc.vector.tensor_tensor(
                            out=t1[:], in0=t1[:], in1=b1_sb[:],
                            op=mybir.AluOpType.add)
                        t2 = fl.tile([128, HID], f32, tag="f2",
                                     name=f"{lname}t2_{g}")
                        nc.scalar.activation(
                            out=t2[:], in_=t1[:],
                            func=mybir.ActivationFunctionType.Relu)
                        nc.vector.tensor_scalar(
                            out=u_own[:, g, :], in0=t2[:], scalar1=dv,
                            scalar2=None, op0=mybir.AluOpType.mult)
                    write_rows(u_own[:, g, :], g)
                else:
                    # self loop (transposed): psumT += u_own[g].T
                    nc.tensor.matmul(out=ps[:], lhsT=u_own[:, g, :],
                                     rhs=ident_sb[:],
                                     start=(g not in first), stop=True)
                    # aggT @ W2, then row-scale by dinv (diagonal commutes)
                    aggT = fl.tile([HID, 128], bf16, tag="f1",
                                   name=f"{lname}aggT_{g}")
                    nc.scalar.activation(
                        out=aggT[:], in_=ps[:],
                        func=mybir.ActivationFunctionType.Copy)
                    o_ps = mps.tile([128, OUT_CH], f32, space="PSUM",
                                    tag="mps", name=f"{lname}ops_{g}")
                    nc.tensor.matmul(out=o_ps[:], lhsT=aggT[:], rhs=w2_sb[:],
                                     start=True, stop=True)
                    o_sb = fl.tile([128, OUT_CH], f32, tag="f3",
                                   name=f"{lname}osb_{g}")
                    if zero_bias:
                        nc.scalar.activation(
                            out=o_sb[:], in_=o_ps[:],
                            func=mybir.ActivationFunctionType.Copy,
                            scale=dinv_sb[:, g:g + 1])
                    else:
                        nc.vector.tensor_scalar(
                            out=o_sb[:], in0=o_ps[:],
                            scalar1=dinv_sb[:, g:g + 1],
                            scalar2=None, op0=mybir.AluOpType.mult)
                        nc.vector.tensor_tensor(
                            out=o_sb[:], in0=o_sb[:], in1=b2_sb[:],
                            op=mybir.AluOpType.add)
                    nc.sync.dma_start(
                        out=out[g * 128:(g + 1) * 128, :], in_=o_sb[:])

            def layer(lname, final, ufull):
                psum = {}
                coloff = 0
                mmoff = 0
                for ci in range(ncalls):
                    ch = ci % NCHUNK
                    ni = sched["nidx_call"][ci]
                    nt = sched["ntile_call"][ci]
                    w16 = sched["idx_cols"][ci]
                    mml = mm_lists[ci]
                    if ni == 0:
                        coloff += w16
                        mmoff += len(mml)
                        continue
                    msg = msgs[ci % MSGBUFS]
                    nc.gpsimd.dma_gather(
                        out_ap=msg[:, 0:nt, :],
                        in_ap=ufull[ch][:],
                        idxs_ap=idx_sb[:, coloff:coloff + w16],
                        num_idxs=ni, num_idxs_reg=ni,
                        elem_size=FEAT, single_packet=False,
                        queue_num=ci % NQUEUES,
                    )
                    nmm_c = len(mml)
                    pm = pp.tile([128, nmm_c, 128], bf16, tag="pmat",
                                 name=f"{lname}pm_{ci}")
                    nc.vector.tensor_tensor(
                        out=pm[:],
                        in0=drel_sb[:, mmoff:mmoff + nmm_c]
                            .to_broadcast([128, nmm_c, 128]),
                        in1=iota_sb[:].unsqueeze(1)
                            .to_broadcast([128, nmm_c, 128]),
                        op=mybir.AluOpType.is_equal,
                    )
                    for j, (t, g) in enumerate(mml):
                        if g not in psum:
                            shape = [HID, 128] if final else [128, HID]
                            psum[g] = gps.tile(shape, f32, space="PSUM",
                                               tag="gacc",
                                               name=f"{lname}acc_{g}")
                        gm = mmoff + j
                        if final:
                            nc.tensor.matmul(
                                out=psum[g][:],
                                lhsT=msg[:, t, 0:HID],
                                rhs=pm[:, j, :],
                                start=(gm == first[g]), stop=False)
                        else:
                            nc.tensor.matmul(
                                out=psum[g][:],
                                lhsT=pm[:, j, :],
                                rhs=msg[:, t, 0:HID],
                                start=(gm == first[g]), stop=False)
                    coloff += w16
                    mmoff += len(mml)
                    # flush groups whose block ends at this call
                    for g in sorted(k for k, v in flush_ci.items() if v == ci):
                        if g not in psum:
                            shape = [HID, 128] if final else [128, HID]
                            psum[g] = gps.tile(shape, f32, space="PSUM",
                                               tag="gacc",
                                               name=f"{lname}acc_{g}")
                        _flush(lname, g, psum.pop(g), final)

            allgather(u_fullA)          # u1
            layer("L1", final=False, ufull=u_fullA)
            allgather(u_fullB)          # u2 (overlaps L1 tail: no WAR on A)
            layer("L2", final=True, ufull=u_fullB)

    nc.compile()
    return nc


_CACHE = {}


def kernel(x, edge_index, W1, b1, W2, b2):
    x = np.asarray(x, np.float32)
    edge_index = np.asarray(edge_index, np.int64)
    sched, in_maps = _host_prep(
        x, edge_index, np.asarray(W1, np.float32), np.asarray(b1, np.float32),
        np.asarray(W2, np.float32), np.asarray(b2, np.float32))
    key = (sched["nmm"], sched["ntiles"], sched["nidx_coltot"],
           sched["zero_bias"])
    if key not in _CACHE:
        _CACHE[key] = _build_program(sched)
    nc = _CACHE[key]
    res = bass_utils.run_bass_kernel_spmd(nc, in_maps,
                                          core_ids=list(range(NCORES)))
    outs = []
    for c in range(NCORES):
        lo = c * SLICE
        hi = min(lo + SLICE, N)
        outs.append(res.results[c]["out"][:hi - lo])
    return np.concatenate(outs, 0).astype(np.float32)



# revision 8
# speedup vs baseline: 2.6450x; 1.0557x over previous
"""GCN 2-layer message passing on 8 Trainium2 NeuronCores.

Strategy (graph/data parallel, hardcoded for N=100000, E=1600000, 128->64->32):
  - Nodes sharded by contiguous range across 8 cores (12544 rows/core, padded).
  - Symmetric normalization folded into per-node row scales (dinv), applied
    pre/post aggregation -> per-edge work is pure gather + segment-sum.
  - Edges owned by dst core, grouped into 128-node dst groups; blocks of 6
    groups x 4 src chunks (int16 gather index limit) form one dma_gather call
    each, UNPADDED (per-call num_idxs = max real edge count across cores; the
    SWDGE descriptor generation on GpSimd is the bottleneck at ~8ns/descriptor,
    so padded descriptors are the thing to avoid).
  - Gathered 128-edge tiles may straddle dst groups: each (tile, group) pair
    present on any core gets its own selection column; P[edge, slot] =
    (dst_rel == iota) built on VectorE (non-members/-1 never match), and
    TensorE matmul-accumulates P.T @ msg into the group's PSUM tile.
    This replaces scatter-add entirely; duplicates merge in PSUM.
  - GCN's added self-loops never enter the edge lists; each group gets one
    identity matmul adding its own rows (kept resident in SBUF).
  - AllGather (collective) re-replicates the per-core node tables for the
    matmul input (u1) and at the layer boundary (u2).
"""
import sys

sys.path.insert(0, "/opt/trn_rl_repo")

import numpy as np
import ml_dtypes

from concourse import bass, mybir
import concourse.bacc as bacc
import concourse.tile as tile
from concourse import bass_utils

BF16 = ml_dtypes.bfloat16

NCORES = 8
N = 100000
IN_CH = 128
HID = 64
OUT_CH = 32
SLICE = 12544          # nodes per core (98 groups of 128)
NPAD = SLICE * NCORES  # 100352
G = SLICE // 128       # 98 groups per core
NCHUNK = 4
CHUNK = NPAD // NCHUNK  # 25088 (< 32768, int16-addressable)
BLOCK = 6              # dst groups per block (PSUM bank budget)
FEAT = 128             # padded bf16 row width of node tables (256B rows)
MSGBUFS = 10
NQUEUES = 4            # SWDGE queues; queue q's desc-gen runs on DSP pair q
PAD_NEG = False  # pad idx tails with -1 (breaks static ring accounting!)


def configure(n):
    """Set problem size (test hook). Recomputes sharding constants."""
    global N, SLICE, NPAD, G, CHUNK
    N = n
    SLICE = -(-N // (NCORES * 128)) * 128
    NPAD = SLICE * NCORES
    G = SLICE // 128
    CHUNK = NPAD // NCHUNK
    assert CHUNK % 16 == 0 and CHUNK < 32768


# ----------------------------------------------------------------------------
# host-side preprocessing: sharding, schedule, index arrays
# ----------------------------------------------------------------------------

def _host_prep(x, edge_index, W1, b1, W2, b2):
    src = edge_index[0].astype(np.int64)
    dst = edge_index[1].astype(np.int64)
    # degree includes the GCN-added self loop (handled on-device as identity)
    deg = (np.bincount(dst, minlength=N) + 1).astype(np.float32)
    dinv = (1.0 / np.sqrt(deg)).astype(np.float32)

    core = (dst // SLICE).astype(np.int64)          # dst owner
    g_loc = ((dst - core * SLICE) // 128).astype(np.int64)
    blk = g_loc // BLOCK
    # src chunk q holds local rows [q*SLICE/4,(q+1)*SLICE/4) of every core,
    # so each chunk is filled by its own (pipelined) AllGather
    qsz = SLICE // NCHUNK
    c_src = src // SLICE
    l_src = src - c_src * SLICE
    ch = l_src // qsz
    nblocks = -(-G // BLOCK)
    call_of = blk * NCHUNK + ch                     # call id within core
    ncalls = nblocks * NCHUNK
    dst_rel = (dst - core * SLICE - g_loc * 128).astype(np.int32)
    idx16 = (c_src * qsz + (l_src - ch * qsz)).astype(np.int16)

    # sort edges by (core, call, group) so group runs are contiguous per call
    key = (core * ncalls + call_of) * G + g_loc
    order = np.argsort(key, kind="stable")
    cc_s = (core * ncalls + call_of)[order]
    g_s = g_loc[order].astype(np.int32)
    idx16_s = idx16[order]
    dstrel_s = dst_rel[order]

    counts = np.bincount(cc_s, minlength=NCORES * ncalls).reshape(
        NCORES, ncalls)
    starts = np.zeros(NCORES * ncalls + 1, np.int64)
    np.cumsum(counts.reshape(-1), out=starts[1:])
    nidx_call = counts.max(axis=0)                  # [ncalls]
    ntile_call = -(-nidx_call // 128)

    # per-core per-call slot arrays (group id, dstrel, idx), padded to max
    # count with (g=-1, dstrel=-1, idx=0)
    mm_lists = []          # per call: ordered [(tile, group), ...]
    for ci in range(ncalls):
        nt = int(ntile_call[ci])
        pairs = set()
        for c in range(NCORES):
            lo, hi = starts[c * ncalls + ci], starts[c * ncalls + ci + 1]
            gs = g_s[lo:hi]
            for t in range(nt):
                for g in np.unique(gs[t * 128:(t + 1) * 128]):
                    pairs.add((t, int(g)))
        mm_lists.append(sorted(pairs))
    nmm = sum(len(m) for m in mm_lists)
    ntiles = int(ntile_call.sum())
    nidx_tot = int(nidx_call.sum())
    idx_cols = [-(-int(n) // 16) for n in nidx_call]
    nidx_coltot = sum(idx_cols)

    idx_w = np.zeros((NCORES, 128, nidx_coltot), np.int16)
    drel_w = np.full((NCORES, 128, nmm), -1.0, np.float32)
    for c in range(NCORES):
        mmoff = 0
        coloff = 0
        for ci in range(ncalls):
            nt = int(ntile_call[ci])
            ncap = nt * 128
            lo, hi = starts[c * ncalls + ci], starts[c * ncalls + ci + 1]
            n = hi - lo
            gs = np.full(ncap, -1, np.int32)
            drs = np.full(ncap, -1.0, np.float32)
            ids = np.full(ncap, -1 if PAD_NEG else 0, np.int16)
            gs[:n] = g_s[lo:hi]
            drs[:n] = dstrel_s[lo:hi]
            ids[:n] = idx16_s[lo:hi]
            # idx wrap for this call: i -> [i%16, i//16], replicated x8
            ni = int(nidx_call[ci])
            w16 = idx_cols[ci]
            blk16 = ids[:w16 * 16].reshape(w16, 16).T
            idx_w[c, :, coloff:coloff + w16] = np.tile(blk16, (8, 1))
            coloff += w16
            # selection columns per (tile, group)
            for j, (t, g) in enumerate(mm_lists[ci]):
                seg_g = gs[t * 128:(t + 1) * 128]
                seg_d = drs[t * 128:(t + 1) * 128]
                drel_w[c, :, mmoff + j] = np.where(seg_g == g, seg_d, -1.0)
            mmoff += len(mm_lists[ci])

    # per-core prescaled transposed features (bf16), zero padded
    xs = x * dinv[:, None]
    xT = np.zeros((NCORES, IN_CH, SLICE), BF16)
    dinv_w = np.zeros((NCORES, 128, G), np.float32)
    dinv2_w = np.zeros((NCORES, 128, G), np.float32)
    for c in range(NCORES):
        lo = c * SLICE
        hi = min(lo + SLICE, N)
        xT[c, :, :hi - lo] = xs[lo:hi].T.astype(BF16)
        dv = np.zeros(SLICE, np.float32)
        dv[:hi - lo] = dinv[lo:hi]
        dinv_w[c] = dv.reshape(G, 128).T
        dinv2_w[c] = (dv * dv).reshape(G, 128).T

    iota = np.tile(np.arange(128, dtype=np.float32), (128, 1)).astype(BF16)
    consts = {
        "w1_in": W1.astype(BF16),                            # [128, 64]
        "w2_in": W2.astype(BF16),                            # [64, 32]
        "b1_in": np.tile(b1.astype(np.float32), (128, 1)),   # [128, 64]
        "b2_in": np.tile(b2.astype(np.float32), (128, 1)),   # [128, 32]
        "iota_in": iota,
        "ident_in": np.eye(128, dtype=np.float32).astype(BF16),
    }
    in_maps = []
    for c in range(NCORES):
        m = dict(consts)
        m["xt_in"] = xT[c]
        m["idx_in"] = idx_w[c]
        m["drel_in"] = drel_w[c].astype(BF16)
        m["dinv_in"] = dinv_w[c]
        m["dinv2_in"] = dinv2_w[c]
        in_maps.append(m)

    sched = {
        "zero_bias": bool(np.all(b1 == 0) and np.all(b2 == 0)),
        "ncalls": ncalls,
        "nidx_call": [int(v) for v in nidx_call],
        "ntile_call": [int(v) for v in ntile_call],
        "idx_cols": idx_cols,
        "mm_lists": mm_lists,
        "nmm": nmm,
        "ntiles": ntiles,
        "nidx_coltot": nidx_coltot,
        "nblocks": nblocks,
    }
    return sched, in_maps


# ----------------------------------------------------------------------------
# device program
# ----------------------------------------------------------------------------

def _build_program(sched):
    f32 = mybir.dt.float32
    bf16 = mybir.dt.bfloat16
    ncalls = sched["ncalls"]
    mm_lists = sched["mm_lists"]
    nmm = sched["nmm"]
    nc = bacc.Bacc("TRN2", target_bir_lowering=False, debug=False,
                   num_devices=NCORES, num_swdge_queues=NQUEUES)

    xt = nc.dram_tensor("xt_in", [IN_CH, SLICE], bf16, kind="ExternalInput").ap()
    idx = nc.dram_tensor("idx_in", [128, sched["nidx_coltot"]], mybir.dt.int16,
                         kind="ExternalInput").ap()
    drel = nc.dram_tensor("drel_in", [128, nmm], bf16,
                          kind="ExternalInput").ap()
    dinv = nc.dram_tensor("dinv_in", [128, G], f32, kind="ExternalInput").ap()
    dinv2 = nc.dram_tensor("dinv2_in", [128, G], f32,
                           kind="ExternalInput").ap()
    w1 = nc.dram_tensor("w1_in", [IN_CH, HID], bf16, kind="ExternalInput").ap()
    w2 = nc.dram_tensor("w2_in", [HID, OUT_CH], bf16, kind="ExternalInput").ap()
    b1 = nc.dram_tensor("b1_in", [128, HID], f32, kind="ExternalInput").ap()
    b2 = nc.dram_tensor("b2_in", [128, OUT_CH], f32, kind="ExternalInput").ap()
    iota_t = nc.dram_tensor("iota_in", [128, 128], bf16,
                            kind="ExternalInput").ap()
    ident = nc.dram_tensor("ident_in", [128, 128], bf16,
                           kind="ExternalInput").ap()
    out = nc.dram_tensor("out", [SLICE, OUT_CH], f32, kind="ExternalOutput").ap()

    # first mm (global index) per group, and flush call per group
    first = {}
    gmm = 0
    for ci in range(ncalls):
        for (t, g) in mm_lists[ci]:
            if g not in first:
                first[g] = gmm
            gmm += 1
    flush_ci = {}
    for g in range(G):
        bi = g // BLOCK
        flush_ci[g] = min((bi + 1) * NCHUNK, ncalls) - 1

    wmax = max(sched["ntile_call"]) if ncalls else 1

    with tile.TileContext(nc) as tc:
        with tc.tile_pool(name="dram", bufs=1, space="DRAM") as dram, \
             tc.tile_pool(name="const", bufs=1) as cst, \
             tc.tile_pool(name="pmat", bufs=4) as pp, \
             tc.tile_pool(name="flush", bufs=3) as fl, \
             tc.tile_pool(name="gpsum", bufs=BLOCK, space="PSUM") as gps, \
             tc.tile_pool(name="mpsum", bufs=2, space="PSUM") as mps:

            # ---- constants / persistent SBUF ----
            idx_sb = cst.tile([128, sched["nidx_coltot"]], mybir.dt.int16)
            nc.sync.dma_start(out=idx_sb[:], in_=idx[:])
            drel_sb = cst.tile([128, nmm], bf16)
            nc.sync.dma_start(out=drel_sb[:], in_=drel[:])
            dinv_sb = cst.tile([128, G], f32)
            nc.sync.dma_start(out=dinv_sb[:], in_=dinv[:])
            dinv2_sb = cst.tile([128, G], f32)
            nc.sync.dma_start(out=dinv2_sb[:], in_=dinv2[:])
            w1_sb = cst.tile([IN_CH, HID], bf16)
            nc.sync.dma_start(out=w1_sb[:], in_=w1[:])
            w2_sb = cst.tile([HID, OUT_CH], bf16)
            nc.sync.dma_start(out=w2_sb[:], in_=w2[:])
            b1_sb = cst.tile([128, HID], f32)
            nc.sync.dma_start(out=b1_sb[:], in_=b1[:])
            b2_sb = cst.tile([128, OUT_CH], f32)
            nc.sync.dma_start(out=b2_sb[:], in_=b2[:])
            iota_sb = cst.tile([128, 128], bf16)
            nc.sync.dma_start(out=iota_sb[:], in_=iota_t[:])
            ident_sb = cst.tile([128, 128], bf16)
            nc.sync.dma_start(out=ident_sb[:], in_=ident[:])
            u_own = cst.tile([128, G, HID], bf16)   # this core's table rows

            # persistent msg buffers (zeroed once: stale tail slots must not
            # hold NaN bit patterns; 0 * garbage-NaN would poison PSUM)
            msgs = []
            for i in range(MSGBUFS):
                mt = cst.tile([128, wmax, FEAT], bf16, name=f"msgbuf{i}")
                nc.vector.memset(mt[:], 0.0)
                msgs.append(mt)

            # DRAM node tables, split into row quarters so each quarter's
            # AllGather starts as soon as its rows are written
            qsz = SLICE // NCHUNK
            u_loc = [dram.tile([qsz, FEAT], bf16, name=f"u_loc{q}")
                     for q in range(NCHUNK)]
            u_fullA = [dram.tile([CHUNK, FEAT], bf16, name=f"u_fullA{q}")
                       for q in range(NCHUNK)]
            u_fullB = [dram.tile([CHUNK, FEAT], bf16, name=f"u_fullB{q}")
                       for q in range(NCHUNK)]

            def write_rows(src_ap, g):
                # DMA u_own[:, g, :]-style tile rows [g*128,(g+1)*128) into
                # the quarter tiles (a group can span two quarters)
                r0 = g * 128
                p = 0
                while p < 128:
                    q = (r0 + p) // qsz
                    take = min(128 - p, (q + 1) * qsz - (r0 + p))
                    nc.sync.dma_start(
                        out=u_loc[q][r0 + p - q * qsz:
                                     r0 + p - q * qsz + take, 0:HID],
                        in_=src_ap[p:p + take])
                    p += take

            # ---- phase A: u1 = (dinv*x) @ W1, local rows ----
            with tc.tile_pool(name="xt", bufs=1) as xtp:
                xt_sb = xtp.tile([IN_CH, SLICE], bf16)
                nc.sync.dma_start(out=xt_sb[:], in_=xt[:])
                for g in range(G):
                    u1_ps = mps.tile([128, HID], f32, space="PSUM",
                                     tag="mps", name=f"u1ps_{g}")
                    nc.tensor.matmul(out=u1_ps[:],
                                     lhsT=xt_sb[:, g * 128:(g + 1) * 128],
                                     rhs=w1_sb[:], start=True, stop=True)
                    nc.scalar.activation(
                        out=u_own[:, g, :], in_=u1_ps[:],
                        func=mybir.ActivationFunctionType.Copy)
                    write_rows(u_own[:, g, :], g)

            def allgather(dst):
                for q in range(NCHUNK):
                    nc.gpsimd.collective_compute(
                        "AllGather", mybir.AluOpType.bypass,
                        replica_groups=[list(range(NCORES))],
                        ins=[u_loc[q][:].opt()], outs=[dst[q][:].opt()],
                    )

            zero_bias = sched["zero_bias"]

            def _flush(lname, g, ps, final):
                if not final:
                    # self loop: psum += I.T @ u_own[g]
                    nc.tensor.matmul(out=ps[:], lhsT=ident_sb[:],
                                     rhs=u_own[:, g, :],
                                     start=(g not in first), stop=True)
                    dv = dinv_sb[:, g:g + 1]
                    if zero_bias:
                        # dinv>0: dinv*relu(dinv*psum) == relu(dinv^2*psum).
                        # One ScalarE op; keeps VectorE free (it stalls badly
                        # against concurrent SWDGE descriptor generation).
                        nc.scalar.activation(
                            out=u_own[:, g, :], in_=ps[:],
                            func=mybir.ActivationFunctionType.Relu,
                            scale=dinv2_sb[:, g:g + 1])
                    else:
                        t1 = fl.tile([128, HID], f32, tag="f1",
                                     name=f"{lname}t1_{g}")
                        nc.vector.tensor_scalar(
                            out=t1[:], in0=ps[:], scalar1=dv, scalar2=None,
                            op0=mybir.AluOpType.mult)
                        nc.vector.tensor_tensor(
                            out=t1[:], in0=t1[:], in1=b1_sb[:],
                            op=mybir.AluOpType.add)
                        t2 = fl.tile([128, HID], f32, tag="f2",
                                     name=f"{lname}t2_{g}")
                        nc.scalar.activation(
                            out=t2[:], in_=t1[:],
                            func=mybir.ActivationFunctionType.Relu)
                        nc.vector.tensor_scalar(
                            out=u_own[:, g, :], in0=t2[:], scalar1=dv,
                            scalar2=None, op0=mybir.AluOpType.mult)
                    write_rows(u_own[:, g, :], g)
                else:
                    # self loop (transposed): psumT += u_own[g].T
                    nc.tensor.matmul(out=ps[:], lhsT=u_own[:, g, :],
                                     rhs=ident_sb[:],
                                     start=(g not in first), stop=True)
                    # aggT @ W2, then row-scale by dinv (diagonal commutes)
                    aggT = fl.tile([HID, 128], bf16, tag="f1",
                                   name=f"{lname}aggT_{g}")
                    nc.scalar.activation(
                        out=aggT[:], in_=ps[:],
                        func=mybir.ActivationFunctionType.Copy)
                    o_ps = mps.tile([128, OUT_CH], f32, space="PSUM",
                                    tag="mps", name=f"{lname}ops_{g}")
                    nc.tensor.matmul(out=o_ps[:], lhsT=aggT[:], rhs=w2_sb[:],
                                     start=True, stop=True)
                    o_sb = fl.tile([128, OUT_CH], f32, tag="f3",
                                   name=f"{lname}osb_{g}")
                    if zero_bias:
                        nc.scalar.activation(
                            out=o_sb[:], in_=o_ps[:],
                            func=mybir.ActivationFunctionType.Copy,
                            scale=dinv_sb[:, g:g + 1])
                    else:
                        nc.vector.tensor_scalar(
                            out=o_sb[:], in0=o_ps[:],
                            scalar1=dinv_sb[:, g:g + 1],
                            scalar2=None, op0=mybir.AluOpType.mult)
                        nc.vector.tensor_tensor(
                            out=o_sb[:], in0=o_sb[:], in1=b2_sb[:],
                            op=mybir.AluOpType.add)
                    nc.sync.dma_start(
                        out=out[g * 128:(g + 1) * 128, :], in_=o_sb[:])

            def layer(lname, final, ufull):
                psum = {}
                coloff = 0
                mmoff = 0
                for ci in range(ncalls):
                    ch = ci % NCHUNK
                    ni = sched["nidx_call"][ci]
                    nt = sched["ntile_call"][ci]
                    w16 = sched["idx_cols"][ci]
                    mml = mm_lists[ci]
                    if ni == 0:
                        coloff += w16
                        mmoff += len(mml)
                        continue
                    msg = msgs[ci % MSGBUFS]
                    nc.gpsimd.dma_gather(
                        out_ap=msg[:, 0:nt, :],
                        in_ap=ufull[ch][:],
                        idxs_ap=idx_sb[:, coloff:coloff + w16],
                        num_idxs=ni, num_idxs_reg=ni,
                        elem_size=FEAT, single_packet=False,
                        queue_num=ci % NQUEUES,
                    )
                    nmm_c = len(mml)
                    pm = pp.tile([128, nmm_c, 128], bf16, tag="pmat",
                                 name=f"{lname}pm_{ci}")
                    nc.vector.tensor_tensor(
                        out=pm[:],
                        in0=drel_sb[:, mmoff:mmoff + nmm_c]
                            .to_broadcast([128, nmm_c, 128]),
                        in1=iota_sb[:].unsqueeze(1)
                            .to_broadcast([128, nmm_c, 128]),
                        op=mybir.AluOpType.is_equal,
                    )
                    for j, (t, g) in enumerate(mml):
                        if g not in psum:
                            shape = [HID, 128] if final else [128, HID]
                            psum[g] = gps.tile(shape, f32, space="PSUM",
                                               tag="gacc",
                                               name=f"{lname}acc_{g}")
                        gm = mmoff + j
                        if final:
                            nc.tensor.matmul(
                                out=psum[g][:],
                                lhsT=msg[:, t, 0:HID],
                                rhs=pm[:, j, :],
                                start=(gm == first[g]), stop=False)
                        else:
                            nc.tensor.matmul(
                                out=psum[g][:],
                                lhsT=pm[:, j, :],
                                rhs=msg[:, t, 0:HID],
                                start=(gm == first[g]), stop=False)
                    coloff += w16
                    mmoff += len(mml)
                    # flush groups whose block ends at this call
                    for g in sorted(k for k, v in flush_ci.items() if v == ci):
                        if g not in psum:
                            shape = [HID, 128] if final else [128, HID]
                            psum[g] = gps.tile(shape, f32, space="PSUM",
                                               tag="gacc",
                                               name=f"{lname}acc_{g}")
                        _flush(lname, g, psum.pop(g), final)

            allgather(u_fullA)          # u1
            layer("L1", final=False, ufull=u_fullA)
            allgather(u_fullB)          # u2 (overlaps L1 tail: no WAR on A)
            layer("L2", final=True, ufull=u_fullB)

    nc.compile()
    return nc


_CACHE = {}


def kernel(x, edge_index, W1, b1, W2, b2):
    x = np.asarray(x, np.float32)
    edge_index = np.asarray(edge_index, np.int64)
    sched, in_maps = _host_prep(
        x, edge_index, np.asarray(W1, np.float32), np.asarray(b1, np.float32),
        np.asarray(W2, np.float32), np.asarray(b2, np.float32))
    key = (sched["nmm"], sched["ntiles"], sched["nidx_coltot"],
           sched["zero_bias"])
    if key not in _CACHE:
        _CACHE[key] = _build_program(sched)
    nc = _CACHE[key]
    res = bass_utils.run_bass_kernel_spmd(nc, in_maps,
                                          core_ids=list(range(NCORES)))
    outs = []
    for c in range(NCORES):
        lo = c * SLICE
        hi = min(lo + SLICE, N)
        outs.append(res.results[c]["out"][:hi - lo])
    return np.concatenate(outs, 0).astype(np.float32)

